# revision 9
# baseline (speedup 1.0000x reference)
"""CoAttLayer Trainium2 kernel — pure data-parallel over batch on 8 NeuronCores.

Reference computation (per batch element b, T=1024, N=512, D=64, K=80):
  L  = tanh(R @ Wl @ P^T)                    (T, N)
  Hp = tanh(Wp @ P^T + (Wr @ R^T) @ L)       (K, N)
  Hr = tanh(Wr @ R^T + (Wp @ P^T) @ L^T)     (K, T)
  Ap = softmax(whp @ Hp), Ar = softmax(whr @ Hr)
  out[b] = concat(P^T @ Ap, R^T @ Ar)        (2D,)

Reassociated into D-sized contractions:
  Hp = [Wp | Wr] @ [P^T ; X]   with X = R^T @ L    (D, N)
  Hr = [Wr | Wp] @ [R^T ; Y]   with Y = P^T @ L^T  (D, T)

Design notes (each validated against perfetto traces / microbenchmarks):
 - All matmul operands bf16 (fp32 PSUM); rel err vs fp32 reference ~5.5e-3.
 - K<=64 matmuls stream moving rows at HALF rate; two K=64 matmuls packed
   into disjoint PE row groups via tile_position run fully concurrently.
   RlT and the L tiles are packed this way, using [Rt;Rt] / [Pt;Pt]
   replicas loaded into both partition halves (the replica halves are
   later overwritten by Y / X — they are only needed before that point).
 - All static transposed layouts (R^T/P^T merged in one tensor, weight
   stacks, wl stacked twice) are prepared on the HOST; zero PE transposes
   for inputs.  The L-pair and RlT PSUM tiles are bf16 (single-shot
   matmuls, no accumulation chains) which halves their PSUM footprint and
   lets the accumulator pool run 4 deep — PSUM-buffer-reuse waits at
   batch boundaries disappear.
 - L^T is made by PE block-transposes of tanh'd L, 8 per L-pair grouped
   into a single 1-bank bf16 PSUM tile with one wide DVE evacuation
   (transposes elsewhere poisoned the PE HAM clock governor; grouped this
   way the surrounding matmul stream keeps the clock up).
 - Each DMA trigger costs ~0.7 us on the in-order Sync sequencer, so
   input loads are merged into 9 whole-tensor DMAs: batch 0 (gating
   compute start) first, all constants in a single bf16 block, then the
   remaining batches merged.
 - Emission interleaves phase2(b-1) (Y/Hr/Hp/logits) before phase1(b)
   (RlT/L/tanh/LT/X) so every cross-engine latency is covered by dense
   independent matmul work; phase2 runs its Y chains first so the final
   batch's drain has the shortest serial chain.
 - Softmax skips the max-subtraction pass (logits are provably small).
   The epilogue is fully per-batch and transpose-free: exp runs directly
   on the [128, 12] transposed-logits tile (partition-major probs are
   exactly the layout pooling wants), the softmax denominators fall out
   of the pooling matmuls via a ones-column appended to the host-packed
   P/R tiles (row 64 of the pooled vector = sum of exp), and the final
   normalization is one reciprocal + per-partition scalar multiply after
   the output transpose.
"""

import numpy as np

import concourse.bass as bass
import concourse.bacc as bacc
import concourse.mybir as mybir
import concourse.tile as tile
from concourse.bass_utils import run_bass_kernel_spmd

F32 = mybir.dt.float32
BF16 = mybir.dt.bfloat16
AF = mybir.ActivationFunctionType

B_LOC = 8      # batch elements per core
T, N, D, K = 1024, 512, 64, 80
TI = T // 128  # 8 t-tiles
NI = N // 128  # 4 n-tiles
DP = D + 1     # packed P/R tiles carry a ones-column for softmax denominators
TN = T + N
NCORES = 8

# constant-block column offsets (single bf16 DMA): ident | wl2 | wt_hp | wt_hr | whp | whr
C_ID, C_WL, C_HP, C_HR, C_WHP, C_WHR, C_TOT = 0, 128, 192, 272, 352, 353, 354


def build_kernel():
    nc = bacc.Bacc("TRN2", debug=False, target_bir_lowering=False)

    ins = {}
    for name, shape, dt in [
        ("review_p", [B_LOC, 128, TI, DP], BF16),
        ("post_p", [B_LOC, 128, NI, DP], BF16),
        ("rp_t", [B_LOC, D, TN], BF16),
        ("cblk", [128, C_TOT], BF16),
    ]:
        ins[name] = nc.declare_dram_parameter(name, shape, dt, isOutput=False)
    out_e = nc.declare_dram_parameter("out", [B_LOC, 2 * D], F32, isOutput=True)

    with tile.TileContext(nc) as tc:
        _body(nc, tc, ins, out_e)

    nc.compile()
    return nc


def _body(nc, tc, ins, out_e):
    from contextlib import ExitStack

    ctx = ExitStack()
    cpool = ctx.enter_context(tc.tile_pool(name="const", bufs=1))
    inpool = ctx.enter_context(tc.tile_pool(name="inputs", bufs=1))
    wk = ctx.enter_context(tc.tile_pool(name="work", bufs=2))
    ps_mm = ctx.enter_context(tc.tile_pool(name="ps_mm", bufs=2, space="PSUM"))
    ps_acc = ctx.enter_context(tc.tile_pool(name="ps_acc", bufs=2, space="PSUM"))

    # HAM warmup: the PE clock gate defaults to 4/8 (1.2 GHz) and needs
    # ~3.4us of sustained matmul activity to open to 8/8 (2.4 GHz).  Burn
    # that window on dummy matmuls over a zeroed scratch tile while the
    # input DMAs are still in flight, so real compute starts warm.
    warm_sb = cpool.tile([128, 512], BF16)
    nc.gpsimd.memset(warm_sb[:], 0.0)
    warm_ps = ps_acc.tile([128, 512], F32, tag="acc", name="warm_ps")
    for w in range(8):
        nc.tensor.matmul(warm_ps[:], warm_sb[:, 0:128], warm_sb[:])

    # ---------------- persistent inputs / DMA plan ----------------
    # rp_all[:, b, 0:T] = R^T replicas (both halves), [:, b, T:TN] = P^T;
    # rows 64:128 of the T / N regions are later overwritten by Y / X.
    rp_all = inpool.tile([128, B_LOC, TN], BF16)
    r_ball = inpool.tile([128, B_LOC, TI, DP], BF16)
    p_ball = inpool.tile([128, B_LOC, NI, DP], BF16)
    cb = cpool.tile([128, C_TOT], BF16)

    rpt_v = ins["rp_t"].ap().rearrange("b d t -> d b t")
    rev_v = ins["review_p"].ap().rearrange("b p i d -> p b i d")
    post_v = ins["post_p"].ap().rearrange("b p j d -> p b j d")
    # batch-0 inputs + constants first (compute gates on them), rest merged
    for lo, hi in ((0, 1), (1, B_LOC)):
        s = slice(lo, hi)
        for h in range(2):
            nc.sync.dma_start(out=rp_all[h * D:(h + 1) * D, s, :], in_=rpt_v[:, s])
        if lo == 0:
            nc.sync.dma_start(out=cb[:], in_=ins["cblk"].ap())
        nc.sync.dma_start(out=r_ball[:, s], in_=rev_v[:, s])
        nc.sync.dma_start(out=p_ball[:, s], in_=post_v[:, s])

    ident_b = cb[:, C_ID:C_ID + 128]
    wl2 = cb[:, C_WL:C_WL + D]
    wt_hp = cb[:, C_HP:C_HP + K]
    wt_hr = cb[:, C_HR:C_HR + K]
    whp_b = cb[0:K, C_WHP:C_WHP + 1]
    whr_b = cb[0:K, C_WHR:C_WHR + 1]
    ident_f = cpool.tile([128, 128], F32)
    nc.vector.tensor_copy(ident_f[:], ident_b[:])

    # Per-batch unnormalized probs (exp of logits), partition-major:
    # cols 0:4 ap n-tiles, 4:12 ar t-tiles — exactly the pooling layout.
    prt_all = inpool.tile([128, B_LOC, 12], BF16)
    # Pooled vectors + softmax denominators: row 0:64 = co vector, row 64
    # = sum of exp; col h=0 post-pool, h=1 review-pool.
    co_sb = inpool.tile([DP, 2, B_LOC], F32)

    # ---------------- main compute, two interleaved phases ----------------
    lt_pool = ctx.enter_context(tc.tile_pool(name="lt", bufs=B_LOC))
    ps_tp = ctx.enter_context(tc.tile_pool(name="ps_tp", bufs=2, space="PSUM"))
    st_all = [dict() for _ in range(B_LOC)]

    def phase1(b):
        st = st_all[b]
        st["hr_in"] = rp_all[:, b, 0:T]
        st["hp_in"] = rp_all[:, b, T:TN]
        st["rlt2"] = wk.tile([128, N], BF16, tag="rlt2", name=f"rlt2{b}")
        l_sb = wk.tile([128, TI, N], BF16, tag="l_sb", name=f"l_sb{b}")
        st["lt_sb"] = lt_pool.tile([128, NI, T], BF16, tag="lt", name=f"lt_sb{b}")
        lps = {}

        # rlt2 layout: top half = RlT chunks 0,2,4,6; bottom = 1,3,5,7,
        # one packed pair with even/odd interleaved views of replicated Rt.
        pss = []
        for h in range(2):
            ps = ps_mm.tile([D, 512], F32, tag="mm", name=f"rlt_ps{b}_{h}")
            rt_v = st["hr_in"][h * D:(h + 1) * D, :].rearrange(
                "p (c two k) -> p two c k", two=2, k=128
            )[:, h]
            nc.tensor.matmul(
                ps[:], wl2[h * D:(h + 1) * D, :], rt_v, tile_position=(h * D, 0)
            )
            pss.append(ps)
        for h in range(2):
            nc.vector.tensor_copy(st["rlt2"][h * D:(h + 1) * D, :], pss[h][:])

        def emit_l_pair(p):
            lp = ps_mm.tile([128, 2, N], F32, tag="mm", name=f"lps{b}_{p}")
            lps[p] = lp
            for h in range(2):
                nc.tensor.matmul(
                    lp[:, h],
                    st["rlt2"][h * D:(h + 1) * D, p * 128:(p + 1) * 128],
                    st["hp_in"][h * D:(h + 1) * D, :],
                    tile_position=(h * D, 0),
                )

        def emit_l_evac(p):
            nc.scalar.activation(l_sb[:, 2 * p:2 * p + 2, :], lps[p][:], AF.Tanh)

        def emit_lt_pair(p):
            # PE block-transposes of the tanh'd pair into one 1-bank PSUM
            # tile, then a single wide DVE evacuation into lt_sb.
            tp = ps_tp.tile([128, NI, 2, 128], BF16, tag="tp", name=f"tp{b}_{p}")
            for j in range(NI):
                for h in range(2):
                    nc.tensor.transpose(
                        tp[:, j, h],
                        l_sb[:, 2 * p + h, j * 128:(j + 1) * 128],
                        ident_b[:],
                    )
            nc.vector.tensor_copy(
                st["lt_sb"][:, :, 2 * p * 128:(2 * p + 2) * 128]
                .rearrange("q j (two k) -> q j two k", k=128),
                tp[:],
            )

        xps = ps_acc.tile([D, N], F32, tag="acc", name=f"xps{b}")
        emit_l_pair(0)
        emit_l_pair(1)
        emit_l_evac(0)
        for p in range(TI // 2):
            for i in (2 * p, 2 * p + 1):
                nc.tensor.matmul(
                    xps[:], r_ball[:, b, i, 0:D], l_sb[:, i],
                    start=(i == 0), stop=(i == TI - 1),
                )
            if p + 2 < TI // 2:
                emit_l_pair(p + 2)
            if p + 1 < TI // 2:
                emit_l_evac(p + 1)
            emit_lt_pair(p)
        nc.vector.tensor_copy(st["hp_in"][D:128, :], xps[:])

    def phase2(b):
        st = st_all[b]
        hp_bf = wk.tile([K, N], BF16, tag="hp_bf", name=f"hp_bf{b}")
        hps = ps_acc.tile([K, N], F32, tag="acc", name=f"hps{b}")
        nc.tensor.matmul(hps[:], wt_hp[:], st["hp_in"][:])
        nc.scalar.activation(hp_bf[:], hps[:], AF.Tanh)

        for c in range(2):
            yps = ps_acc.tile([D, 512], F32, tag="acc", name=f"yps{b}_{c}")
            for j in range(NI):
                nc.tensor.matmul(
                    yps[:], p_ball[:, b, j, 0:D],
                    st["lt_sb"][:, j, c * 512:(c + 1) * 512],
                    start=(j == 0), stop=(j == NI - 1),
                )
            nc.vector.tensor_copy(
                st["hr_in"][D:128, c * 512:(c + 1) * 512], yps[:]
            )

        hr_bf = wk.tile([K, T], BF16, tag="hr_bf", name=f"hr_bf{b}")
        for c in range(2):
            hrs = ps_acc.tile([K, 512], F32, tag="acc", name=f"hrs{b}_{c}")
            nc.tensor.matmul(hrs[:], wt_hr[:], st["hr_in"][:, c * 512:(c + 1) * 512])
            nc.scalar.activation(hr_bf[:, c * 512:(c + 1) * 512], hrs[:], AF.Tanh)

        lg_ps = ps_acc.tile([128, 12], F32, tag="acc", name=f"lg_ps{b}")
        for j in range(NI):
            nc.tensor.matmul(
                lg_ps[:, j:j + 1], hp_bf[:, j * 128:(j + 1) * 128], whp_b[:],
                skip_group_check=True,
            )
        for i in range(TI):
            nc.tensor.matmul(
                lg_ps[:, 4 + i:5 + i], hr_bf[:, i * 128:(i + 1) * 128], whr_b[:],
                skip_group_check=True,
            )
        # logits are bounded (|whp|_1-weighted tanh values): exp cannot
        # overflow, so softmax runs without the max-subtraction pass.
        # Unnormalized probs go straight to the pooling layout.
        nc.scalar.activation(prt_all[:, b, :], lg_ps[:], AF.Exp)

    def epilogue(b):
        # Attention-weighted pooling, 1-moving-row matmuls accumulated in
        # PSUM.  The stationary P/R tiles carry a ones-column (col 64), so
        # row 64 of the pooled vector is the softmax denominator.
        co_ps = ps_acc.tile([DP, 2], F32, tag="acc", name=f"co_ps{b}")
        for j in range(NI):
            nc.tensor.matmul(
                co_ps[:, 0:1], p_ball[:, b, j], prt_all[:, b, j:j + 1],
                start=(j == 0), stop=(j == NI - 1), skip_group_check=True,
            )
        for i in range(TI):
            nc.tensor.matmul(
                co_ps[:, 1:2], r_ball[:, b, i],
                prt_all[:, b, 4 + i:5 + i],
                start=(i == 0), stop=(i == TI - 1), skip_group_check=True,
            )
        nc.vector.tensor_copy(co_sb[:, :, b], co_ps[:])

    # Two-deep software pipeline: phase1(b+1) is emitted BEFORE phase2(b-1)
    # consumes lt_sb(b-1), giving the DVE a full phase1 of slack to finish
    # the L^T evacuations — the Y chains never wait on them.
    phase1(0)
    phase1(1)
    for b in range(1, B_LOC):
        phase2(b - 1)
        if b + 1 < B_LOC:
            phase1(b + 1)
        epilogue(b - 1)
    phase2(B_LOC - 1)
    epilogue(B_LOC - 1)

    # Transpose (65, 16) -> (16, 65); row h*8+b is the h-half of out[b],
    # col 64 the denominator.  Normalize with one per-partition multiply.
    cot_ps = ps_acc.tile([2 * B_LOC, DP], F32, tag="acc")
    nc.tensor.transpose(
        cot_ps[:], co_sb[:].rearrange("d h b -> d (h b)"), ident_f[0:DP, 0:DP]
    )
    rcp16 = inpool.tile([2 * B_LOC, 1], F32)
    nc.vector.reciprocal(rcp16[:], cot_ps[:, D:DP])
    out_sb = inpool.tile([2 * B_LOC, D], F32)
    nc.vector.tensor_scalar_mul(out_sb[:], cot_ps[:, 0:D], rcp16[:, 0:1])
    nc.sync.dma_start(out=out_e.ap()[:, 0:D], in_=out_sb[0:B_LOC, :])
    nc.sync.dma_start(out=out_e.ap()[:, D:2 * D], in_=out_sb[B_LOC:2 * B_LOC, :])
    ctx.close()


_NC_CACHE = None


def _get_nc():
    global _NC_CACHE
    if _NC_CACHE is None:
        _NC_CACHE = build_kernel()
    return _NC_CACHE


def _prep_host_inputs(inputs):
    import ml_dtypes

    bf = ml_dtypes.bfloat16
    rev = np.ascontiguousarray(np.asarray(inputs["review_seq"], dtype=np.float32))
    post = np.ascontiguousarray(np.asarray(inputs["post_seq"], dtype=np.float32))
    wl = np.asarray(inputs["Wl"], dtype=np.float32)
    wr = np.asarray(inputs["Wr"], dtype=np.float32)
    wp = np.asarray(inputs["Wp"], dtype=np.float32)
    whr = np.asarray(inputs["whr"], dtype=np.float32)
    whp = np.asarray(inputs["whp"], dtype=np.float32)

    rev_bf = rev.astype(bf)
    post_bf = post.astype(bf)
    B = rev.shape[0]
    # packed layouts [b, p, tile, d] with a ones-column at d=64 (softmax
    # denominator trick); column order matches t = 8p + i / n = 4p + j.
    rev_p = np.concatenate(
        [rev_bf.reshape(B, 128, TI, D), np.ones((B, 128, TI, 1), dtype=bf)],
        axis=-1,
    )
    post_p = np.concatenate(
        [post_bf.reshape(B, 128, NI, D), np.ones((B, 128, NI, 1), dtype=bf)],
        axis=-1,
    )
    # rp_t[b, d, i*128 + p] = rev[b, 8p + i, d]; cols T: = same for post
    rev_t = rev_bf.reshape(B, 128, TI, D).transpose(0, 3, 2, 1).reshape(B, D, T)
    post_t = post_bf.reshape(B, 128, NI, D).transpose(0, 3, 2, 1).reshape(B, D, N)
    rp_t = np.ascontiguousarray(np.concatenate([rev_t, post_t], axis=2))

    cblk = np.zeros((128, C_TOT), dtype=bf)
    cblk[:, C_ID:C_ID + 128] = np.eye(128, dtype=bf)
    cblk[:, C_WL:C_WL + D] = np.concatenate([wl, wl], axis=0).astype(bf)
    cblk[:, C_HP:C_HP + K] = np.concatenate([wp.T, wr.T], axis=0).astype(bf)
    cblk[:, C_HR:C_HR + K] = np.concatenate([wr.T, wp.T], axis=0).astype(bf)
    cblk[0:K, C_WHP:C_WHP + 1] = whp.T.astype(bf)
    cblk[0:K, C_WHR:C_WHR + 1] = whr.T.astype(bf)
    return rev_p, post_p, rp_t, cblk


def run_on_hw(inputs: dict, trace: bool = False, **kw):
    nc = _get_nc()
    rev_p, post_p, rp_t, cblk = _prep_host_inputs(inputs)
    in_maps = []
    for c in range(NCORES):
        s = slice(c * B_LOC, (c + 1) * B_LOC)
        m = {
            "review_p": np.ascontiguousarray(rev_p[s]),
            "post_p": np.ascontiguousarray(post_p[s]),
            "rp_t": np.ascontiguousarray(rp_t[s]),
            "cblk": cblk,
        }
        in_maps.append(m)
    res = run_bass_kernel_spmd(nc, in_maps, list(range(NCORES)), trace=trace, **kw)
    out = np.concatenate([res.results[c]["out"] for c in range(NCORES)], axis=0)
    return out, res


def kernel(**inputs) -> np.ndarray:
    out, _ = run_on_hw(inputs, trace=False)
    return out.astype(np.float32)


# revision 10
# speedup vs baseline: 1.0557x; 1.0557x over previous
"""CoAttLayer Trainium2 kernel — pure data-parallel over batch on 8 NeuronCores.

Reference computation (per batch element b, T=1024, N=512, D=64, K=80):
  L  = tanh(R @ Wl @ P^T)                    (T, N)
  Hp = tanh(Wp @ P^T + (Wr @ R^T) @ L)       (K, N)
  Hr = tanh(Wr @ R^T + (Wp @ P^T) @ L^T)     (K, T)
  Ap = softmax(whp @ Hp), Ar = softmax(whr @ Hr)
  out[b] = concat(P^T @ Ap, R^T @ Ar)        (2D,)

Reassociated into D-sized contractions:
  Hp = [Wp | Wr] @ [P^T ; X]   with X = R^T @ L    (D, N)
  Hr = [Wr | Wp] @ [R^T ; Y]   with Y = P^T @ L^T  (D, T)

Design notes (each validated against perfetto traces / microbenchmarks):
 - All matmul operands bf16 (fp32 PSUM); rel err vs fp32 reference ~5.5e-3.
 - K<=64 matmuls stream moving rows at HALF rate; two K=64 matmuls packed
   into disjoint PE row groups via tile_position run fully concurrently.
   RlT and the L tiles are packed this way, using [Rt;Rt] / [Pt;Pt]
   replicas loaded into both partition halves (the replica halves are
   later overwritten by Y / X — they are only needed before that point).
 - All static transposed layouts (R^T/P^T merged in one tensor, weight
   stacks, wl stacked twice) are prepared on the HOST; zero PE transposes
   for inputs.  The L-pair and RlT PSUM tiles are bf16 (single-shot
   matmuls, no accumulation chains) which halves their PSUM footprint and
   lets the accumulator pool run 4 deep — PSUM-buffer-reuse waits at
   batch boundaries disappear.
 - L^T is made by PE block-transposes of tanh'd L, 8 per L-pair grouped
   into a single 1-bank bf16 PSUM tile with one wide DVE evacuation
   (transposes elsewhere poisoned the PE HAM clock governor; grouped this
   way the surrounding matmul stream keeps the clock up).
 - Each DMA trigger costs ~0.7 us on the in-order Sync sequencer, so
   input loads are merged into 9 whole-tensor DMAs: batch 0 (gating
   compute start) first, all constants in a single bf16 block, then the
   remaining batches merged.
 - Emission interleaves phase2(b-1) (Y/Hr/Hp/logits) before phase1(b)
   (RlT/L/tanh/LT/X) so every cross-engine latency is covered by dense
   independent matmul work; phase2 runs its Y chains first so the final
   batch's drain has the shortest serial chain.
 - Softmax skips the max-subtraction pass (logits are provably small).
   The epilogue is fully per-batch and transpose-free: exp runs directly
   on the [128, 12] transposed-logits tile (partition-major probs are
   exactly the layout pooling wants), the softmax denominators fall out
   of the pooling matmuls via a ones-column appended to the host-packed
   P/R tiles (row 64 of the pooled vector = sum of exp), and the final
   normalization is one reciprocal + per-partition scalar multiply after
   the output transpose.
"""

import numpy as np

import concourse.bass as bass
import concourse.bacc as bacc
import concourse.mybir as mybir
import concourse.tile as tile
from concourse.bass_utils import run_bass_kernel_spmd

F32 = mybir.dt.float32
BF16 = mybir.dt.bfloat16
AF = mybir.ActivationFunctionType

B_LOC = 8      # batch elements per core
T, N, D, K = 1024, 512, 64, 80
TI = T // 128  # 8 t-tiles
NI = N // 128  # 4 n-tiles
DP = D + 1     # packed P/R tiles carry a ones-column for softmax denominators
TN = T + N
NCORES = 8

# constant-block column offsets (single bf16 DMA): ident | wl2 | wt_hp | wt_hr | whp | whr
C_ID, C_WL, C_HP, C_HR, C_WHP, C_WHR, C_TOT = 0, 128, 192, 272, 352, 353, 354


def build_kernel():
    nc = bacc.Bacc("TRN2", debug=False, target_bir_lowering=False)

    ins = {}
    for name, shape, dt in [
        ("review_p", [B_LOC, 128, TI, DP], BF16),
        ("post_p", [B_LOC, 128, NI, DP], BF16),
        ("rp_t", [B_LOC, D, TN], BF16),
        ("cblk", [128, C_TOT], BF16),
    ]:
        ins[name] = nc.declare_dram_parameter(name, shape, dt, isOutput=False)
    out_e = nc.declare_dram_parameter("out", [B_LOC, 2 * D], F32, isOutput=True)

    with tile.TileContext(nc) as tc:
        _body(nc, tc, ins, out_e)

    nc.compile()
    return nc


def _body(nc, tc, ins, out_e):
    from contextlib import ExitStack

    ctx = ExitStack()
    cpool = ctx.enter_context(tc.tile_pool(name="const", bufs=1))
    inpool = ctx.enter_context(tc.tile_pool(name="inputs", bufs=1))
    wk = ctx.enter_context(tc.tile_pool(name="work", bufs=2))
    ps_mm = ctx.enter_context(tc.tile_pool(name="ps_mm", bufs=2, space="PSUM"))
    ps_acc = ctx.enter_context(tc.tile_pool(name="ps_acc", bufs=2, space="PSUM"))

    # HAM warmup: the PE clock gate defaults to 4/8 (1.2 GHz) and needs
    # ~3.4us of sustained matmul activity to open to 8/8 (2.4 GHz).  Burn
    # that window on dummy matmuls over a zeroed scratch tile while the
    # input DMAs are still in flight, so real compute starts warm.
    warm_sb = cpool.tile([128, 512], BF16)
    nc.gpsimd.memset(warm_sb[:], 0.0)
    warm_ps = ps_acc.tile([128, 512], F32, tag="acc", name="warm_ps")
    for w in range(8):
        nc.tensor.matmul(warm_ps[:], warm_sb[:, 0:128], warm_sb[:])

    # ---------------- persistent inputs / DMA plan ----------------
    # rp_all[:, b, 0:T] = R^T replicas (both halves), [:, b, T:TN] = P^T;
    # rows 64:128 of the T / N regions are later overwritten by Y / X.
    rp_all = inpool.tile([128, B_LOC, TN], BF16)
    r_ball = inpool.tile([128, B_LOC, TI, DP], BF16)
    p_ball = inpool.tile([128, B_LOC, NI, DP], BF16)
    cb = cpool.tile([128, C_TOT], BF16)

    rpt_v = ins["rp_t"].ap().rearrange("b d t -> d b t")
    rev_v = ins["review_p"].ap().rearrange("b p i d -> p b i d")
    post_v = ins["post_p"].ap().rearrange("b p j d -> p b j d")
    # batch-0 inputs + constants first (compute gates on them), rest merged
    for lo, hi in ((0, 1), (1, B_LOC)):
        s = slice(lo, hi)
        for h in range(2):
            nc.sync.dma_start(out=rp_all[h * D:(h + 1) * D, s, :], in_=rpt_v[:, s])
        if lo == 0:
            nc.sync.dma_start(out=cb[:], in_=ins["cblk"].ap())
        nc.sync.dma_start(out=r_ball[:, s], in_=rev_v[:, s])
        nc.sync.dma_start(out=p_ball[:, s], in_=post_v[:, s])

    ident_b = cb[:, C_ID:C_ID + 128]
    wl2 = cb[:, C_WL:C_WL + D]
    wt_hp = cb[:, C_HP:C_HP + K]
    wt_hr = cb[:, C_HR:C_HR + K]
    whp_b = cb[0:K, C_WHP:C_WHP + 1]
    whr_b = cb[0:K, C_WHR:C_WHR + 1]
    ident_f = cpool.tile([128, 128], F32)
    nc.vector.tensor_copy(ident_f[:], ident_b[:])

    # Per-batch unnormalized probs (exp of logits), partition-major:
    # cols 0:4 ap n-tiles, 4:12 ar t-tiles — exactly the pooling layout.
    prt_all = inpool.tile([128, B_LOC, 12], BF16)
    # Pooled vectors + softmax denominators: row 0:64 = co vector, row 64
    # = sum of exp; col h=0 post-pool, h=1 review-pool.
    co_sb = inpool.tile([DP, 2, B_LOC], F32)

    # ---------------- main compute, two interleaved phases ----------------
    lt_pool = ctx.enter_context(tc.tile_pool(name="lt", bufs=B_LOC))
    ps_tp = ctx.enter_context(tc.tile_pool(name="ps_tp", bufs=2, space="PSUM"))
    st_all = [dict() for _ in range(B_LOC)]

    def phase1(b):
        st = st_all[b]
        st["hr_in"] = rp_all[:, b, 0:T]
        st["hp_in"] = rp_all[:, b, T:TN]
        st["rlt2"] = wk.tile([128, N], BF16, tag="rlt2", name=f"rlt2{b}")
        l_sb = wk.tile([128, TI, N], BF16, tag="l_sb", name=f"l_sb{b}")
        st["lt_sb"] = lt_pool.tile([128, NI, T], BF16, tag="lt", name=f"lt_sb{b}")
        lps = {}

        # rlt2 layout: top half = RlT chunks 0,2,4,6; bottom = 1,3,5,7,
        # one packed pair with even/odd interleaved views of replicated Rt.
        pss = []
        for h in range(2):
            ps = ps_mm.tile([D, 512], F32, tag="mm", name=f"rlt_ps{b}_{h}")
            rt_v = st["hr_in"][h * D:(h + 1) * D, :].rearrange(
                "p (c two k) -> p two c k", two=2, k=128
            )[:, h]
            nc.tensor.matmul(
                ps[:], wl2[h * D:(h + 1) * D, :], rt_v, tile_position=(h * D, 0)
            )
            pss.append(ps)
        for h in range(2):
            nc.vector.tensor_copy(st["rlt2"][h * D:(h + 1) * D, :], pss[h][:])

        def emit_l_pair(p):
            lp = ps_mm.tile([128, 2, N], F32, tag="mm", name=f"lps{b}_{p}")
            lps[p] = lp
            for h in range(2):
                nc.tensor.matmul(
                    lp[:, h],
                    st["rlt2"][h * D:(h + 1) * D, p * 128:(p + 1) * 128],
                    st["hp_in"][h * D:(h + 1) * D, :],
                    tile_position=(h * D, 0),
                )

        def emit_l_evac(p):
            nc.scalar.activation(l_sb[:, 2 * p:2 * p + 2, :], lps[p][:], AF.Tanh)

        def emit_lt_pair(p):
            # PE block-transposes of the tanh'd pair into one 1-bank PSUM
            # tile, then a single wide DVE evacuation into lt_sb.
            tp = ps_tp.tile([128, NI, 2, 128], BF16, tag="tp", name=f"tp{b}_{p}")
            for j in range(NI):
                for h in range(2):
                    nc.tensor.transpose(
                        tp[:, j, h],
                        l_sb[:, 2 * p + h, j * 128:(j + 1) * 128],
                        ident_b[:],
                    )
            nc.vector.tensor_copy(
                st["lt_sb"][:, :, 2 * p * 128:(2 * p + 2) * 128]
                .rearrange("q j (two k) -> q j two k", k=128),
                tp[:],
            )

        xps = ps_acc.tile([D, N], F32, tag="acc", name=f"xps{b}")
        emit_l_pair(0)
        emit_l_pair(1)
        emit_l_evac(0)
        for p in range(TI // 2):
            for i in (2 * p, 2 * p + 1):
                nc.tensor.matmul(
                    xps[:], r_ball[:, b, i, 0:D], l_sb[:, i],
                    start=(i == 0), stop=(i == TI - 1),
                )
            if p + 2 < TI // 2:
                emit_l_pair(p + 2)
            if p + 1 < TI // 2:
                emit_l_evac(p + 1)
            emit_lt_pair(p)
        nc.vector.tensor_copy(st["hp_in"][D:128, :], xps[:])

    def phase2(b):
        st = st_all[b]
        hp_bf = wk.tile([K, N], BF16, tag="hp_bf", name=f"hp_bf{b}")
        hps = ps_acc.tile([K, N], F32, tag="acc", name=f"hps{b}")
        nc.tensor.matmul(hps[:], wt_hp[:], st["hp_in"][:])
        nc.scalar.activation(hp_bf[:], hps[:], AF.Tanh)

        for c in range(2):
            yps = ps_acc.tile([D, 512], F32, tag="acc", name=f"yps{b}_{c}")
            for j in range(NI):
                nc.tensor.matmul(
                    yps[:], p_ball[:, b, j, 0:D],
                    st["lt_sb"][:, j, c * 512:(c + 1) * 512],
                    start=(j == 0), stop=(j == NI - 1),
                )
            nc.vector.tensor_copy(
                st["hr_in"][D:128, c * 512:(c + 1) * 512], yps[:]
            )

        hr_bf = wk.tile([K, T], BF16, tag="hr_bf", name=f"hr_bf{b}")
        for c in range(2):
            hrs = ps_acc.tile([K, 512], F32, tag="acc", name=f"hrs{b}_{c}")
            nc.tensor.matmul(hrs[:], wt_hr[:], st["hr_in"][:, c * 512:(c + 1) * 512])
            nc.scalar.activation(hr_bf[:, c * 512:(c + 1) * 512], hrs[:], AF.Tanh)

        lg_ps = ps_acc.tile([128, 12], F32, tag="acc", name=f"lg_ps{b}")
        for j in range(NI):
            nc.tensor.matmul(
                lg_ps[:, j:j + 1], hp_bf[:, j * 128:(j + 1) * 128], whp_b[:],
                skip_group_check=True,
            )
        for i in range(TI):
            nc.tensor.matmul(
                lg_ps[:, 4 + i:5 + i], hr_bf[:, i * 128:(i + 1) * 128], whr_b[:],
                skip_group_check=True,
            )
        # logits are bounded (|whp|_1-weighted tanh values): exp cannot
        # overflow, so softmax runs without the max-subtraction pass.
        # Unnormalized probs go straight to the pooling layout.
        nc.scalar.activation(prt_all[:, b, :], lg_ps[:], AF.Exp)

    def epilogue(b):
        # Attention-weighted pooling, 1-moving-row matmuls accumulated in
        # PSUM.  The stationary P/R tiles carry a ones-column (col 64), so
        # row 64 of the pooled vector is the softmax denominator.
        co_ps = ps_acc.tile([DP, 2], F32, tag="acc", name=f"co_ps{b}")
        for j in range(NI):
            nc.tensor.matmul(
                co_ps[:, 0:1], p_ball[:, b, j], prt_all[:, b, j:j + 1],
                start=(j == 0), stop=(j == NI - 1), skip_group_check=True,
            )
        for i in range(TI):
            nc.tensor.matmul(
                co_ps[:, 1:2], r_ball[:, b, i],
                prt_all[:, b, 4 + i:5 + i],
                start=(i == 0), stop=(i == TI - 1), skip_group_check=True,
            )
        nc.vector.tensor_copy(co_sb[:, :, b], co_ps[:])

    phase1(0)
    for b in range(1, B_LOC):
        phase2(b - 1)
        phase1(b)
        epilogue(b - 1)
    phase2(B_LOC - 1)
    epilogue(B_LOC - 1)

    # Transpose (65, 16) -> (16, 65); row h*8+b is the h-half of out[b],
    # col 64 the denominator.  Normalize with one per-partition multiply.
    cot_ps = ps_acc.tile([2 * B_LOC, DP], F32, tag="acc")
    nc.tensor.transpose(
        cot_ps[:], co_sb[:].rearrange("d h b -> d (h b)"), ident_f[0:DP, 0:DP]
    )
    rcp16 = inpool.tile([2 * B_LOC, 1], F32)
    nc.vector.reciprocal(rcp16[:], cot_ps[:, D:DP])
    out_sb = inpool.tile([2 * B_LOC, D], F32)
    nc.vector.tensor_scalar_mul(out_sb[:], cot_ps[:, 0:D], rcp16[:, 0:1])
    nc.sync.dma_start(out=out_e.ap()[:, 0:D], in_=out_sb[0:B_LOC, :])
    nc.sync.dma_start(out=out_e.ap()[:, D:2 * D], in_=out_sb[B_LOC:2 * B_LOC, :])
    ctx.close()


_NC_CACHE = None


def _get_nc():
    global _NC_CACHE
    if _NC_CACHE is None:
        _NC_CACHE = build_kernel()
    return _NC_CACHE


def _prep_host_inputs(inputs):
    import ml_dtypes

    bf = ml_dtypes.bfloat16
    rev = np.ascontiguousarray(np.asarray(inputs["review_seq"], dtype=np.float32))
    post = np.ascontiguousarray(np.asarray(inputs["post_seq"], dtype=np.float32))
    wl = np.asarray(inputs["Wl"], dtype=np.float32)
    wr = np.asarray(inputs["Wr"], dtype=np.float32)
    wp = np.asarray(inputs["Wp"], dtype=np.float32)
    whr = np.asarray(inputs["whr"], dtype=np.float32)
    whp = np.asarray(inputs["whp"], dtype=np.float32)

    rev_bf = rev.astype(bf)
    post_bf = post.astype(bf)
    B = rev.shape[0]
    # packed layouts [b, p, tile, d] with a ones-column at d=64 (softmax
    # denominator trick); column order matches t = 8p + i / n = 4p + j.
    rev_p = np.concatenate(
        [rev_bf.reshape(B, 128, TI, D), np.ones((B, 128, TI, 1), dtype=bf)],
        axis=-1,
    )
    post_p = np.concatenate(
        [post_bf.reshape(B, 128, NI, D), np.ones((B, 128, NI, 1), dtype=bf)],
        axis=-1,
    )
    # rp_t[b, d, i*128 + p] = rev[b, 8p + i, d]; cols T: = same for post
    rev_t = rev_bf.reshape(B, 128, TI, D).transpose(0, 3, 2, 1).reshape(B, D, T)
    post_t = post_bf.reshape(B, 128, NI, D).transpose(0, 3, 2, 1).reshape(B, D, N)
    rp_t = np.ascontiguousarray(np.concatenate([rev_t, post_t], axis=2))

    cblk = np.zeros((128, C_TOT), dtype=bf)
    cblk[:, C_ID:C_ID + 128] = np.eye(128, dtype=bf)
    cblk[:, C_WL:C_WL + D] = np.concatenate([wl, wl], axis=0).astype(bf)
    cblk[:, C_HP:C_HP + K] = np.concatenate([wp.T, wr.T], axis=0).astype(bf)
    cblk[:, C_HR:C_HR + K] = np.concatenate([wr.T, wp.T], axis=0).astype(bf)
    cblk[0:K, C_WHP:C_WHP + 1] = whp.T.astype(bf)
    cblk[0:K, C_WHR:C_WHR + 1] = whr.T.astype(bf)
    return rev_p, post_p, rp_t, cblk


def run_on_hw(inputs: dict, trace: bool = False, **kw):
    nc = _get_nc()
    rev_p, post_p, rp_t, cblk = _prep_host_inputs(inputs)
    in_maps = []
    for c in range(NCORES):
        s = slice(c * B_LOC, (c + 1) * B_LOC)
        m = {
            "review_p": np.ascontiguousarray(rev_p[s]),
            "post_p": np.ascontiguousarray(post_p[s]),
            "rp_t": np.ascontiguousarray(rp_t[s]),
            "cblk": cblk,
        }
        in_maps.append(m)
    res = run_bass_kernel_spmd(nc, in_maps, list(range(NCORES)), trace=trace, **kw)
    out = np.concatenate([res.results[c]["out"] for c in range(NCORES)], axis=0)
    return out, res


def kernel(**inputs) -> np.ndarray:
    out, _ = run_on_hw(inputs, trace=False)
    return out.astype(np.float32)
